# revision 12
# baseline (speedup 1.0000x reference)
"""DiffPool forward Trainium2 kernel (8-core SPMD, Bass/Tile).

Strategy (data-parallel over graphs, per the sharding hint):
  - 1024 graphs -> 128 graphs per core. Nodes of each graph are padded to a
    uniform G slots (G = max graph size rounded up to 64) so that every core
    runs the same instruction stream with static graph/block geometry.
  - Per 128-node block: PE transposes x, fp32 matmul vs W^T -> logits in PSUM;
    DVE adds gumbel, row-max, one-hot via is_equal (assignment == y_hard
    because y_hard + y_soft - stop_grad(y_soft) == y_hard to 1 ulp);
    cluster index via reduce_max(one_hot * iota).
  - segment_sum: per padded graph, accumulate one_hot^T @ x in PSUM across its
    (static) partition-range pieces, evacuate to the output shard.
  - edge relabel idx[edge_list]: edge endpoints are sharded by the owning
    graph's core and sorted by node. The device broadcasts idx into a DRAM
    table bc64[slot, 64], then dma_gather expands it row-wise (each row = one
    node repeated 64x); the host scatters the gathered values back to the
    original edge positions (pure unshard permutation, precomputed from
    edge_list/node2graph only).
"""
import numpy as np

import concourse.bass as bass
import concourse.bacc as bacc
import concourse.tile as tile
import concourse.mybir as mybir
from concourse import bass_utils
from concourse.masks import make_identity

P = 128
K = 64
D = 256
NCORES = 8
SBB = 8            # blocks per superblock
CHUNK = 8192       # dma_gather rows per call
SLICE = 8192       # bc64 slot rows per slice tensor (finer => earlier gather overlap)

_CACHE = {}


def _build_piece_map(nblocks, G, ngraph):
    pieces = []
    for blk in range(nblocks):
        lo, hi = blk * P, (blk + 1) * P
        out = []
        for g in range(max(0, lo // G), min(ngraph, hi // G + 1)):
            s, e = max(lo, g * G), min(hi, (g + 1) * G)
            if s < e:
                out.append((g, s - lo, e - lo, s == g * G, e == (g + 1) * G))
        pieces.append(out)
    return pieces


def build_program(G, bias_nonzero, ablate=()):
    """Build the single-core Bass program (shared by all 8 cores)."""
    NPAD = P * G               # 128 graphs * G slots
    NB = NPAD // P
    NSB = NB // SBB
    NGRAPH = NPAD // G
    PIECES = _build_piece_map(NB, G, NGRAPH)
    fp32 = mybir.dt.float32

    nc = bacc.Bacc("TRN2", target_bir_lowering=False, debug=False,
                   num_devices=NCORES)
    x_pad = nc.dram_tensor("x_pad", [NPAD, D], fp32, kind="ExternalInput").ap()
    g_pack = nc.dram_tensor("g_pack", [NSB, P, SBB, K], fp32, kind="ExternalInput").ap()
    wt = nc.dram_tensor("wt", [D, K], fp32, kind="ExternalInput").ap()
    iotaf = nc.dram_tensor("iotaf", [P, K], fp32, kind="ExternalInput").ap()
    if bias_nonzero:
        brep = nc.dram_tensor("brep", [1, K], fp32, kind="ExternalInput").ap()
    assign_pack = nc.dram_tensor("assign_pack", [NSB, P, SBB, K], fp32,
                                 kind="ExternalOutput").ap()
    out_shard = nc.dram_tensor("out_shard", [NGRAPH * K, D], fp32,
                               kind="ExternalOutput").ap()
    exp_full = nc.dram_tensor("exp_full", [NPAD, K], fp32,
                              kind="ExternalOutput").ap()

    with tile.TileContext(nc) as tc:
        with (
            tc.tile_pool(name="const", bufs=1) as cpool,
            tc.tile_pool(name="xb", bufs=2 + SBB) as xpool,
            tc.tile_pool(name="xt", bufs=3) as xtpool,
            tc.tile_pool(name="sbw", bufs=2) as sbw,
            tc.tile_pool(name="small", bufs=3) as small,
            tc.tile_pool(name="psum", bufs=2, space="PSUM") as pp,
            tc.tile_pool(name="psumL", bufs=2, space="PSUM") as ppL,
            tc.tile_pool(name="psumG", bufs=2, space="PSUM") as ppG,
            tc.tile_pool(name="outb", bufs=3) as outbp,
            tc.tile_pool(name="dram", bufs=1, space="DRAM") as dpool,
        ):
            ident = cpool.tile([P, P], fp32)
            make_identity(nc, ident[:])
            wtile = cpool.tile([P, 2, K], fp32)
            nc.sync.dma_start(wtile[:, 0, :], wt[0:P, :])
            nc.sync.dma_start(wtile[:, 1, :], wt[P:D, :])
            iota_f = cpool.tile([P, K], fp32)
            nc.sync.dma_start(iota_f[:], iotaf[:])
            if bias_nonzero:
                ones1 = cpool.tile([1, P], fp32)
                nc.vector.memset(ones1[:], 1.0)
                btile = cpool.tile([1, K], fp32)
                nc.sync.dma_start(btile[:], brep[:])

            pass

            pg_live = {}
            for sb in range(NSB):
                gum = sbw.tile([P, SBB, K], fp32, tag="gum")
                nc.sync.dma_start(gum[:], g_pack[sb])
                psumL = ppL.tile([P, SBB, K], fp32)
                xsb = xpool.tile([P, SBB, D], fp32, tag="xsb")
                nc.sync.dma_start(
                    xsb[:],
                    x_pad[sb * P * SBB:(sb + 1) * P * SBB, :].rearrange(
                        "(b p) d -> p b d", p=P))
                for b0 in range(0, SBB, 2):
                    psumT = pp.tile([P, 2 * D], fp32)
                    if "trans" not in ablate:
                        for j in range(2):
                            xb = xsb[:, b0 + j, :]
                            nc.tensor.transpose(
                                psumT[:, j * D:j * D + P], xb[:, 0:P], ident[:])
                            nc.tensor.transpose(
                                psumT[:, j * D + P:(j + 1) * D], xb[:, P:D],
                                ident[:])
                    xt = xtpool.tile([P, 4, P], fp32)
                    if "evac" not in ablate:
                        nc.scalar.copy(xt[:], psumT[:])
                    for j in range(2):
                        b = b0 + j
                        for c in range(2 if "logits" not in ablate else 0):
                            nc.tensor.matmul(
                                out=psumL[:, b, :], lhsT=xt[:, 2 * j + c, :],
                                rhs=wtile[:, c, :], start=(c == 0),
                                stop=(c == 1 and not bias_nonzero))
                        if bias_nonzero:
                            nc.tensor.matmul(
                                out=psumL[:, b, :], lhsT=ones1[:], rhs=btile[:],
                                start=False, stop=True)
                if "dve" in ablate:
                    continue
                zb = sbw.tile([P, SBB, K], fp32, tag="zb")
                nc.vector.tensor_tensor(out=zb[:], in0=psumL[:], in1=gum[:],
                                        op=mybir.AluOpType.add)
                mb = small.tile([P, SBB], fp32, tag="mb")
                nc.vector.reduce_max(out=mb[:], in_=zb[:],
                                     axis=mybir.AxisListType.X)
                yb = sbw.tile([P, SBB, K], fp32, tag="yb")
                nc.vector.tensor_tensor(out=yb[:], in0=zb[:],
                                        in1=mb[:].to_broadcast([P, SBB, K]),
                                        op=mybir.AluOpType.is_equal)
                if "assign" not in ablate:
                    nc.sync.dma_start(assign_pack[sb], yb[:])
                tb = sbw.tile([P, SBB, K], fp32, tag="tb")
                iota_bc = bass.AP(
                    tensor=iota_f.tensor, offset=0,
                    ap=[[iota_f[:].ap[0][0], P], [0, SBB], [1, K]])
                nc.vector.tensor_tensor(out=tb[:], in0=yb[:], in1=iota_bc,
                                        op=mybir.AluOpType.mult)
                idxf = small.tile([P, SBB], fp32, tag="idxf")
                nc.vector.reduce_max(out=idxf[:], in_=tb[:],
                                     axis=mybir.AxisListType.X)
                # idx broadcast 64-wide, then to DRAM bc64[slot] rows
                if "bc" in ablate:
                    continue
                bct = sbw.tile([P, SBB, K], fp32, tag="bct")
                nc.gpsimd.tensor_copy(bct[:], idxf[:].to_broadcast([P, SBB, K]))
                nc.sync.dma_start(
                    exp_full[sb * P * SBB:(sb + 1) * P * SBB, :].rearrange(
                        "(b p) k -> p b k", p=P),
                    bct[:])
                # scatter matmuls
                for b in range(SBB):
                    if "scatter" in ablate:
                        break
                    blk = sb * SBB + b
                    for (g, p0, p1, first, last) in PIECES[blk]:
                        if first:
                            pg_live[g] = ppG.tile([K, D], fp32,
                                                  tag=f"g{g % 2}",
                                                  name=f"pg{g}")
                        pg = pg_live[g]
                        nc.tensor.matmul(
                            out=pg[:], lhsT=yb[p0:p1, b, :],
                            rhs=xsb[p0:p1, b, :], start=first, stop=last)
                        if last:
                            if g % 2 == 0:
                                ob_live = outbp.tile([2 * K, D], fp32,
                                                     tag="ob", name=f"ob{g}")
                                nc.vector.tensor_copy(ob_live[0:K, :], pg[:])
                                pending_ob = ob_live
                            else:
                                nc.scalar.copy(pending_ob[K:2 * K, :], pg[:])
                                nc.sync.dma_start(
                                    out_shard[(g - 1) * K:(g + 1) * K, :],
                                    pending_ob[:])
                            del pg_live[g]

    nc.compile()
    return nc


def _prep(x, W, b, gumbel, edge_list, node2graph, batch_size):
    N = x.shape[0]
    E2 = edge_list.size
    batch = int(batch_size)
    ngraph_core = batch // NCORES
    gstart = np.searchsorted(node2graph, np.arange(batch + 1)).astype(np.int64)
    gsize = np.diff(gstart)
    G = int(max(64, -(-int(gsize.max()) // 64) * 64))
    NPAD = P * G

    nodes = np.arange(N, dtype=np.int64)
    gid = node2graph.astype(np.int64)
    core_of_node = gid // ngraph_core
    slot_of_node = (gid % ngraph_core) * G + (nodes - gstart[gid])

    # padded & packed per-core inputs
    x_pad = np.zeros((NCORES, NPAD, D), np.float32)
    x_pad[core_of_node, slot_of_node] = x
    gum_pad = np.zeros((NCORES, NPAD, K), np.float32)
    gum_pad[core_of_node, slot_of_node] = gumbel
    NSB = NPAD // (P * SBB)
    g_pack = np.ascontiguousarray(
        gum_pad.reshape(NCORES, NSB, SBB, P, K).transpose(0, 1, 3, 2, 4))

    # ---- edge refs: per-ref (core, slot, rank); device outputs a 64-wide
    # broadcast of idx per slot, host scatters back by (slot, rank % 64) ----
    refs = edge_list.reshape(-1).astype(np.int64)
    rcore = core_of_node[refs]
    rslot = slot_of_node[refs]
    order = np.lexsort((np.arange(refs.size), rslot, rcore))
    sorted_keys = rcore[order] * NPAD + rslot[order]
    is_new = np.r_[True, sorted_keys[1:] != sorted_keys[:-1]]
    first_idx = np.flatnonzero(is_new)
    group = np.cumsum(is_new) - 1
    rank = np.arange(refs.size) - first_idx[group]
    take_src = (rcore[order] * (NPAD * K) + rslot[order] * K + rank % K)
    take_dst = order

    meta = dict(G=G, NPAD=NPAD, NSB=NSB,
                core_of_node=core_of_node, slot_of_node=slot_of_node,
                take_src=take_src, take_dst=take_dst,
                ngraph_core=ngraph_core)
    ins = dict(x_pad=x_pad, g_pack=g_pack,
               wt=np.ascontiguousarray(W.T.astype(np.float32)))
    return meta, ins


def kernel(x, W, b, gumbel, edge_list, node2graph, batch_size):
    x = np.asarray(x); W = np.asarray(W); b = np.asarray(b)
    gumbel = np.asarray(gumbel)
    edge_list = np.asarray(edge_list); node2graph = np.asarray(node2graph)
    meta, ins = _prep(x, W, b, gumbel, edge_list, node2graph, batch_size)
    bias_nonzero = bool(np.any(b))

    key = (meta["G"], bias_nonzero)
    if key not in _CACHE:
        _CACHE[key] = build_program(meta["G"], bias_nonzero)
    nc = _CACHE[key]

    in_maps = []
    for c in range(NCORES):
        m = {"x_pad": ins["x_pad"][c], "g_pack": ins["g_pack"][c],
             "wt": ins["wt"],
             "iotaf": np.tile(np.arange(K, dtype=np.float32), (P, 1))}
        if bias_nonzero:
            m["brep"] = b.astype(np.float32).reshape(1, K)
        in_maps.append(m)

    res = bass_utils.run_bass_kernel_spmd(nc, in_maps,
                                          core_ids=list(range(NCORES)))
    results = res.results

    # ---- unshard ----
    N = x.shape[0]
    NSB, K_, P_ = meta["NSB"], K, P
    assign_pad = np.stack([
        results[c]["assign_pack"].transpose(0, 2, 1, 3).reshape(meta["NPAD"], K)
        for c in range(NCORES)])
    assignment = assign_pad[meta["core_of_node"], meta["slot_of_node"]]

    output = np.concatenate([results[c]["out_shard"] for c in range(NCORES)],
                            axis=0)

    exp_all = np.stack([results[c]["exp_full"] for c in range(NCORES)])
    new_edge_flat = np.empty(edge_list.size, np.int32)
    new_edge_flat[meta["take_dst"]] = exp_all.reshape(-1)[meta["take_src"]].astype(np.int32)
    new_edge_list = new_edge_flat.reshape(edge_list.shape)

    return (new_edge_list, output, assignment)
